# revision 36
# baseline (speedup 1.0000x reference)
"""MixLoRA sparse-MoE Trainium2 kernel, v2.

Tensor-parallel over d_ff (F=4096 -> FC=512 per core) on 8 NeuronCores; every
core computes all N=1024 tokens for its F-slice, partial outputs are summed on
the host (bf16 partials).

Speed strategy vs v1:
- Base gate/up GEMMs and LoRA-A GEMMs run as fp8e4 DoubleRow matmuls with a
  3-term split-precision scheme (x ~ xh+xl, w ~ wh+wl, drop xl*wl): K=256 per
  instruction at 0.5 cycles/row -> 0.75x the fp32r cycle count. hi/lo fp8
  operands are produced on the host (weights scaled by 64 to avoid fp8
  subnormals; PSUM carries 64x values, extraction rescales by 1/64).
- Router stays exact: f32r matmul with HOST-REPLICATED gate weights [D, ER]
  producing logits already replicated 16x per expert, so the top-2 masks come
  straight from partition_all_reduce(max) + is_equal with no replicate matmul.
  Top-2 renormalized weights via sigmoid(m1-m2) identity.
- All element-wise intermediates are bf16 in SBUF: DVE runs at 2x; LoRA-B /
  down / A2 / B2 matmuls take bf16 operands (1 cycle/row, same as f32r).
- Unit structure is "full extraction": base PSUM groups are extracted to SBUF
  immediately (frees PSUM banks so bases stream before the router resolves);
  per-branch LoRA-B deltas land in their own PSUM and are combined with the
  base via fused PSUM+SBUF adds on DVE/Pool.
- Output partials are bf16 (half the outbound DMA).
"""
import sys

sys.path.insert(0, "/opt/trn_rl_repo")

from contextlib import ExitStack

import ml_dtypes
import numpy as np

import concourse.tile as tile
from concourse import bacc, bass_isa, mybir
from concourse.bass_utils import run_bass_kernel_spmd

f32 = mybir.dt.float32
f32r = mybir.dt.float32r
bf16 = mybir.dt.bfloat16
f8 = mybir.dt.float8e4
AF = mybir.ActivationFunctionType
ALU = mybir.AluOpType
RED = bass_isa.ReduceOp
PM = mybir.MatmulPerfMode

NCORES = 8
N = 1024          # tokens (B*S)
D = 1024          # hidden
F = 4096          # d_ff
FC = F // NCORES  # 512 per-core f-slice
E = 8             # experts
R = 16            # lora rank
ER = E * R        # 128
NT = 512          # token tile
P = 128
DT = D // P       # 8
KP = DT // 2      # 4 fp8 k-tile pairs
FT = FC // P      # 4
TT = N // NT      # 2
WS = 64.0         # fp8 weight pre-scale
IWS = 1.0 / WS

F8NP = ml_dtypes.float8_e4m3
BFNP = ml_dtypes.bfloat16

_CACHE = {}


def _build():
    nc = bacc.Bacc("TRN2", target_bir_lowering=False, debug=False)

    x8_d = nc.dram_tensor("x8", [KP, 4, P, N], f8, kind="ExternalInput")
    a8_d = nc.dram_tensor("a8", [D, 4, ER], f8, kind="ExternalInput")
    wblk_d = nc.dram_tensor("wblk", [FT, D, 4, P], f8, kind="ExternalInput")
    gwr_d = nc.dram_tensor("gwr", [D, ER], f32, kind="ExternalInput")
    xT_d = nc.dram_tensor("xT", [D, N], f32, kind="ExternalInput")
    bts_d = nc.dram_tensor("bts", [ER, 2, FC], bf16, kind="ExternalInput")
    wa2_d = nc.dram_tensor("wa2", [FC, D + ER], bf16, kind="ExternalInput")
    b2f_d = nc.dram_tensor("b2f", [ER, D], bf16, kind="ExternalInput")
    outT_d = nc.dram_tensor("outT", [D, N], bf16, kind="ExternalOutput")

    with tile.TileContext(nc) as tc:
      with ExitStack() as ctx:
        sb = ctx.enter_context(tc.tile_pool(name="sb", bufs=1))
        ps = ctx.enter_context(tc.tile_pool(name="ps", bufs=2, space="PSUM"))
        psB = ctx.enter_context(tc.tile_pool(name="psB", bufs=1, space="PSUM"))
        work = ctx.enter_context(tc.tile_pool(name="work", bufs=2))
        cpool = ctx.enter_context(tc.tile_pool(name="cpool", bufs=10))
        opool = ctx.enter_context(tc.tile_pool(name="opool", bufs=5))

        # ---------------- persistent tiles ----------------
        x8 = sb.tile([P, KP, 4, N], f8)       # (hi d0, hi d1, lo d0, lo d1)
        a8 = sb.tile([P, DT, 4 * ER], f8)     # a1h a1l a3h a3l columns
        wblk = sb.tile([P, FT, DT, 4 * P], f8)  # per ft: w1h w1l w3h w3l
        gwr = sb.tile([P, DT, ER], f32r)
        xT = sb.tile([P, DT, N], f32r)
        bts = sb.tile([ER, 2, FC], bf16)      # b1t, b3t (true scale, x2)
        wa2 = sb.tile([P, FT, D + ER], bf16)  # wdt | a2t
        b2f = sb.tile([ER, D], bf16)

        s1b = sb.tile([ER, 1, N], bf16)
        s3b = sb.tile([ER, 1, N], bf16)
        mk = sb.tile([ER, 2, N], bf16)
        m1p = sb.tile([ER, 2, N], bf16)
        m3p = sb.tile([ER, 2, N], bf16)
        wa_bc = sb.tile([P, N], bf16)
        wb_bc = sb.tile([P, N], bf16)
        actC = sb.tile([P, FT, N], bf16)
        zc = sb.tile([ER, N], bf16)
        c1f = sb.tile([P, TT, FT, NT], bf16)  # held base extractions
        c3f = sb.tile([P, TT, FT, NT], bf16)

        # ---------------- DMA issue (order = serial bus order) -------------
        x8_src = x8_d[:, :, :, :].rearrange("k c p w -> p k c w")
        a8_src = a8_d[:, :, :].rearrange("(a p) c w -> p a (c w)", p=P)
        wblk_src = wblk_d[:, :, :, :].rearrange(
            "f (a p) c w -> p f a (c w)", p=P)
        gwr_src = gwr_d[:, :].rearrange("(a p) w -> p a w", p=P).bitcast(f32r)
        xT_src = xT_d[:, :].rearrange("(a p) w -> p a w", p=P).bitcast(f32r)

        # all input DMAs on the sync queue: its program order IS the serial
        # bus order, so data arrival is fully controlled here
        dma = nc.sync.dma_start
        dma(out=a8[:, 0:4, :], in_=a8_src[:, 0:4, :])
        dma(out=x8[:, 0, :, :], in_=x8_src[:, 0, :, :])
        dma(out=x8[:, 1, :, :], in_=x8_src[:, 1, :, :])
        dma(out=a8[:, 4:DT, :], in_=a8_src[:, 4:DT, :])
        dma(out=x8[:, 2, :, :], in_=x8_src[:, 2, :, :])
        dma(out=x8[:, 3, :, :], in_=x8_src[:, 3, :, :])
        dma(out=wblk[:, 0, :, :], in_=wblk_src[:, 0, :, :])
        dma(out=wblk[:, 1, :, :], in_=wblk_src[:, 1, :, :])
        dma(out=gwr[:], in_=gwr_src)
        dma(out=bts[:], in_=bts_d[:, :, :])
        for dt_ in range(DT):
            dma(out=xT[:, dt_, 0:NT], in_=xT_src[:, dt_, 0:NT])
        dma(out=wblk[:, 2, :, :], in_=wblk_src[:, 2, :, :])
        dma(out=wblk[:, 3, :, :], in_=wblk_src[:, 3, :, :])
        for dt_ in range(DT):
            dma(out=xT[:, dt_, NT:N], in_=xT_src[:, dt_, NT:N])
        dma(out=wa2[:], in_=wa2_d[:, :].rearrange("(a p) w -> p a w", p=P))
        dma(out=b2f[:], in_=b2f_d[:, :])

        # ---------------- helpers ----------------
        def mm3_group(pout, lh_hi, lh_lo, tsl):
            """12 DR matmuls: 3-term fp8 contraction over D into pout."""
            first = True
            for kp in range(KP):
                xh_sl = x8[:, kp, 0:2, tsl]
                xl_sl = x8[:, kp, 2:4, tsl]
                for lh, rh in ((lh_hi, xh_sl), (lh_hi, xl_sl),
                               (lh_lo, xh_sl)):
                    nc.tensor.matmul(
                        out=pout[:], lhsT=lh[kp], rhs=rh,
                        start=first, stop=(kp == KP - 1 and lh is lh_lo),
                        perf_mode=PM.DoubleRow)
                    first = False

        def a_lh(col):
            """A-stage lhsT slices per kpair: a8 col in {0:a1h,1:a1l,2:a3h,3:a3l}"""
            return [a8[:, 2 * kp:2 * kp + 2, col * ER:(col + 1) * ER]
                    for kp in range(KP)]

        def w_lh(ft, col):
            return [wblk[:, ft, 2 * kp:2 * kp + 2, col * P:(col + 1) * P]
                    for kp in range(KP)]

        # ======== emission units ========
        def emit_A(tt):
            tsl = slice(tt * NT, (tt + 1) * NT)
            pS1 = psB.tile([ER, NT], f32, tag="D1")
            mm3_group(pS1, a_lh(0), a_lh(1), tsl)
            nc.scalar.activation(out=s1b[:, 0, tsl], in_=pS1[:], func=AF.Copy,
                                 scale=IWS)
            pS3 = psB.tile([ER, NT], f32, tag="D3")
            mm3_group(pS3, a_lh(2), a_lh(3), tsl)
            nc.scalar.activation(out=s3b[:, 0, tsl], in_=pS3[:], func=AF.Copy,
                                 scale=IWS)

        def emit_base(tt, ft):
            tsl = slice(tt * NT, (tt + 1) * NT)
            pX = ps.tile([P, NT], f32, tag="X")
            mm3_group(pX, w_lh(ft, 0), w_lh(ft, 1), tsl)
            nc.scalar.activation(out=c1f[:, tt, ft, :], in_=pX[:],
                                 func=AF.Copy, scale=IWS)
            pY = ps.tile([P, NT], f32, tag="Y")
            mm3_group(pY, w_lh(ft, 2), w_lh(ft, 3), tsl)
            nc.scalar.activation(out=c3f[:, tt, ft, :], in_=pY[:],
                                 func=AF.Copy, scale=IWS)

        def emit_router(tt):
            tsl = slice(tt * NT, (tt + 1) * NT)
            pR = ps.tile([ER, NT], f32, tag="X")
            for dt_ in range(DT):
                nc.tensor.matmul(out=pR[:], lhsT=gwr[:, dt_, :],
                                 rhs=xT[:, dt_, tsl],
                                 start=(dt_ == 0), stop=(dt_ == DT - 1))
            lgf = work.tile([ER, NT], f32, tag="lgf")
            nc.scalar.copy(out=lgf[:], in_=pR[:])
            m1 = work.tile([ER, NT], f32, tag="m1")
            nc.gpsimd.partition_all_reduce(m1[:], lgf[:], channels=ER,
                                           reduce_op=RED.max)
            nc.vector.tensor_tensor(out=mk[:, 0, tsl], in0=lgf[:], in1=m1[:],
                                    op=ALU.is_equal)
            l2 = work.tile([ER, NT], f32, tag="lgf2")
            nc.vector.scalar_tensor_tensor(
                out=l2[:], in0=mk[:, 0, tsl], scalar=-1e30, in1=lgf[:],
                op0=ALU.mult, op1=ALU.add)
            m2 = work.tile([ER, NT], f32, tag="m2")
            nc.gpsimd.partition_all_reduce(m2[:], l2[:], channels=ER,
                                           reduce_op=RED.max)
            nc.vector.tensor_tensor(out=mk[:, 1, tsl], in0=l2[:], in1=m2[:],
                                    op=ALU.is_equal)
            d12 = work.tile([1, NT], f32, tag="d12")
            nc.vector.tensor_tensor(out=d12[:], in0=m1[0:1, :],
                                    in1=m2[0:1, :], op=ALU.subtract)
            wa1 = work.tile([1, NT], bf16, tag="wa1")
            nc.scalar.activation(out=wa1[:], in_=d12[:], func=AF.Sigmoid)
            wb1 = work.tile([1, NT], bf16, tag="wb1")
            nc.scalar.activation(out=wb1[:], in_=d12[:], func=AF.Sigmoid,
                                 scale=-1.0)
            nc.gpsimd.partition_broadcast(wa_bc[:, tsl], wa1[:])
            nc.gpsimd.partition_broadcast(wb_bc[:, tsl], wb1[:])
            # masked s for the B-stage
            nc.vector.tensor_tensor(
                out=m1p[:, :, tsl],
                in0=s1b[:, :, tsl].broadcast_to([ER, 2, NT]),
                in1=mk[:, :, tsl], op=ALU.mult)
            nc.vector.tensor_tensor(
                out=m3p[:, :, tsl],
                in0=s3b[:, :, tsl].broadcast_to([ER, 2, NT]),
                in1=mk[:, :, tsl], op=ALU.mult)

        ca_tiles = {}
        cb_tiles = {}

        def emit_unit(tt, ft):
            """Mask-dependent half of a unit: B-deltas, silu chain, actC,
            and streamed z accumulation (pZA/pZB) per ft."""
            tsl = slice(tt * NT, (tt + 1) * NT)
            fsl = slice(ft * P, (ft + 1) * P)
            p1 = psB.tile([P, 2, NT], f32, tag="D1")
            nc.tensor.matmul(out=p1[:, 0, :], lhsT=bts[:, 0, fsl],
                             rhs=m1p[:, 0, tsl], start=True, stop=True)
            nc.tensor.matmul(out=p1[:, 1, :], lhsT=bts[:, 0, fsl],
                             rhs=m1p[:, 1, tsl], start=True, stop=True)
            x1 = work.tile([P, 2, NT], bf16, tag="x1")
            nc.vector.tensor_tensor(
                out=x1[:], in0=p1[:],
                in1=c1f[:, tt, ft:ft + 1, :].broadcast_to([P, 2, NT]),
                op=ALU.add)
            p3 = psB.tile([P, 2, NT], f32, tag="D3")
            nc.tensor.matmul(out=p3[:, 0, :], lhsT=bts[:, 1, fsl],
                             rhs=m3p[:, 0, tsl], start=True, stop=True)
            nc.tensor.matmul(out=p3[:, 1, :], lhsT=bts[:, 1, fsl],
                             rhs=m3p[:, 1, tsl], start=True, stop=True)
            ua = work.tile([P, NT], bf16, tag="ua")
            nc.scalar.activation(out=ua[:], in_=x1[:, 0, :], func=AF.Silu)
            ub = work.tile([P, NT], bf16, tag="ub")
            nc.scalar.activation(out=ub[:], in_=x1[:, 1, :], func=AF.Silu)
            x3 = work.tile([P, 2, NT], bf16, tag="x3")
            nc.vector.tensor_tensor(
                out=x3[:], in0=p3[:],
                in1=c3f[:, tt, ft:ft + 1, :].broadcast_to([P, 2, NT]),
                op=ALU.add)
            xa3 = x3[:, 0, :]
            xb3 = x3[:, 1, :]
            # weight-muls off the silu critical path (Pool)
            xaw = work.tile([P, NT], bf16, tag="xaw")
            nc.gpsimd.tensor_tensor(out=xaw[:], in0=xa3,
                                    in1=wa_bc[:, tsl], op=ALU.mult)
            v = work.tile([P, NT], bf16, tag="v")
            nc.vector.tensor_tensor(out=v[:], in0=ub[:], in1=wb_bc[:, tsl],
                                    op=ALU.mult)
            ca = cpool.tile([P, NT], bf16, tag="ca")
            nc.vector.tensor_tensor(out=ca[:], in0=ua[:], in1=xaw[:],
                                    op=ALU.mult)
            cb = cpool.tile([P, NT], bf16, tag="cb")
            nc.vector.tensor_tensor(out=cb[:], in0=v[:], in1=xb3,
                                    op=ALU.mult)
            nc.gpsimd.tensor_tensor(out=actC[:, ft, tsl], in0=ca[:],
                                    in1=cb[:], op=ALU.add)
            ca_tiles[(ft, tt)] = ca
            cb_tiles[(ft, tt)] = cb

        def emit_z(tt):
            tsl = slice(tt * NT, (tt + 1) * NT)
            pZA = psB.tile([ER, NT], f32, tag="D1")
            for ft in range(FT):
                nc.tensor.matmul(out=pZA[:], lhsT=wa2[:, ft, D:D + ER],
                                 rhs=ca_tiles[(ft, tt)][:],
                                 start=(ft == 0), stop=(ft == FT - 1))
            pZB = psB.tile([ER, NT], f32, tag="D3")
            for ft in range(FT):
                nc.tensor.matmul(out=pZB[:], lhsT=wa2[:, ft, D:D + ER],
                                 rhs=cb_tiles[(ft, tt)][:],
                                 start=(ft == 0), stop=(ft == FT - 1))
            za = cpool.tile([ER, NT], bf16, tag="ca")
            nc.vector.tensor_tensor(out=za[:], in0=pZA[:],
                                    in1=mk[:, 0, tsl], op=ALU.mult)
            zb = cpool.tile([ER, NT], bf16, tag="cb")
            nc.vector.tensor_tensor(out=zb[:], in0=pZB[:],
                                    in1=mk[:, 1, tsl], op=ALU.mult)
            nc.gpsimd.tensor_tensor(out=zc[:, tsl], in0=za[:], in1=zb[:],
                                     op=ALU.add)

        def emit_down(tt, dts):
            tsl = slice(tt * NT, (tt + 1) * NT)
            for dpair in dts:
                ost = opool.tile([P, 2, NT], bf16, tag="ot")
                for j in range(2):
                    dt_ = 2 * dpair + j
                    po = ps.tile([P, NT], f32,
                                 tag=("X" if j == 0 else "Y"))
                    for ft in range(FT):
                        nc.tensor.matmul(
                            out=po[:],
                            lhsT=wa2[:, ft, dt_ * P:(dt_ + 1) * P],
                            rhs=actC[:, ft, tsl],
                            start=(ft == 0), stop=False)
                    nc.tensor.matmul(out=po[:],
                                     lhsT=b2f[:, dt_ * P:(dt_ + 1) * P],
                                     rhs=zc[:, tsl], start=False, stop=True)
                    nc.scalar.copy(out=ost[:, j, :], in_=po[:])
                oeng = nc.sync
                out_view = outT_d[2 * dpair * P:(2 * dpair + 2) * P, tsl]
                oeng.dma_start(
                    out=out_view.rearrange("(a p) w -> p a w", p=P),
                    in_=ost[:])

        # ================= global emission order =================
        emit_A(0)
        emit_A(1)
        emit_base(0, 0)
        emit_base(1, 0)
        emit_base(0, 1)
        emit_base(1, 1)
        emit_router(0)
        emit_unit(0, 0)
        emit_base(0, 2)
        emit_unit(0, 1)
        emit_base(1, 2)
        emit_unit(0, 2)
        emit_base(0, 3)
        emit_unit(0, 3)
        emit_base(1, 3)
        emit_router(1)
        emit_z(0)
        emit_unit(1, 0)
        emit_unit(1, 1)
        emit_unit(1, 2)
        emit_down(0, [0, 1])
        emit_unit(1, 3)
        emit_down(0, [2, 3])
        emit_z(1)
        emit_down(1, [0, 1, 2, 3])

    nc.compile()
    return nc


def _q8(a):
    return np.asarray(a, dtype=F8NP)


def _split8(a, scale=1.0):
    s = (a * scale).astype(np.float32)
    hi = _q8(s)
    lo = _q8(s - hi.astype(np.float32))
    return hi, lo


def _prep_in_maps(inputs):
    hs = np.asarray(inputs["hidden_states"], dtype=np.float32)
    gate_w = np.asarray(inputs["gate_w"], dtype=np.float32)
    w_gate = np.asarray(inputs["w_gate"], dtype=np.float32)
    w_up = np.asarray(inputs["w_up"], dtype=np.float32)
    w_down = np.asarray(inputs["w_down"], dtype=np.float32)
    A1 = np.asarray(inputs["A1"], dtype=np.float32)
    B1 = np.asarray(inputs["B1"], dtype=np.float32)
    A3 = np.asarray(inputs["A3"], dtype=np.float32)
    B3 = np.asarray(inputs["B3"], dtype=np.float32)
    A2 = np.asarray(inputs["A2"], dtype=np.float32)
    B2 = np.asarray(inputs["B2"], dtype=np.float32)

    x = hs.reshape(-1, D)
    C = np.ascontiguousarray
    xT = C(x.T)                      # [D, N]
    xh, xl = _split8(xT)

    # x8 [KP, 4, P, N]: (hi d0, hi d1, lo d0, lo d1) per kpair
    x8 = np.empty((KP, 4, P, N), dtype=F8NP)
    xh3 = xh.reshape(DT, P, N)
    xl3 = xl.reshape(DT, P, N)
    for kp in range(KP):
        x8[kp, 0] = xh3[2 * kp]
        x8[kp, 1] = xh3[2 * kp + 1]
        x8[kp, 2] = xl3[2 * kp]
        x8[kp, 3] = xl3[2 * kp + 1]

    # a8 [D, 4, ER]
    a1h, a1l = _split8(A1.reshape(ER, D).T, WS)
    a3h, a3l = _split8(A3.reshape(ER, D).T, WS)
    a8 = np.stack([a1h, a1l, a3h, a3l], axis=1)

    gwr = C(np.repeat(gate_w, R, axis=0).T)   # [D, ER] f32

    bf = lambda a: np.asarray(a, dtype=BFNP)

    in_maps = []
    for c in range(NCORES):
        fsl = slice(c * FC, (c + 1) * FC)
        w1h, w1l = _split8(w_gate[fsl].T, WS)   # [D, FC]
        w3h, w3l = _split8(w_up[fsl].T, WS)
        wblk = np.empty((FT, D, 4, P), dtype=F8NP)
        for ft in range(FT):
            cs = slice(ft * P, (ft + 1) * P)
            wblk[ft, :, 0] = w1h[:, cs]
            wblk[ft, :, 1] = w1l[:, cs]
            wblk[ft, :, 2] = w3h[:, cs]
            wblk[ft, :, 3] = w3l[:, cs]
        bts = np.stack([
            bf((2.0 * B1[:, fsl, :]).transpose(0, 2, 1).reshape(ER, FC)),
            bf((2.0 * B3[:, fsl, :]).transpose(0, 2, 1).reshape(ER, FC)),
        ], axis=1)                               # [ER, 2, FC]
        wa2 = np.concatenate([
            bf(w_down[:, fsl].T),                # [FC, D]
            bf(A2[:, :, fsl].reshape(ER, FC).T),  # [FC, ER]
        ], axis=1)                               # [FC, D+ER]
        in_maps.append({
            "x8": x8,
            "a8": a8,
            "wblk": wblk,
            "gwr": gwr,
            "xT": xT,
            "bts": C(bts),
            "wa2": C(wa2),
            "b2f": bf((2.0 * B2).transpose(0, 2, 1).reshape(ER, D)),
        })
    return in_maps, hs.shape


def kernel(**inputs):
    if "nc" not in _CACHE:
        _CACHE["nc"] = _build()
    nc = _CACHE["nc"]
    in_maps, (B, S, _) = _prep_in_maps(inputs)
    res = run_bass_kernel_spmd(nc, in_maps, list(range(NCORES)))
    acc = np.zeros((D, N), dtype=np.float32)
    for c in range(NCORES):
        acc += res.results[c]["outT"].astype(np.float32)
    return np.ascontiguousarray(acc.T).reshape(B, S, D)
